# revision 28
# baseline (speedup 1.0000x reference)
"""Distributed Trainium2 kernel for GQA attention (B=2, T=2048, D=2048, N=8
query heads, K=1 KV head, H=256) on 8 NeuronCores.

Sharding: 2 (batch) x 4 (head-pair) mesh. Core c handles batch c//4 and query
heads {2*(c%4), 2*(c%4)+1}. K/V projections are computed per batch group
(replicated across the 4 cores of a group). The per-head out-projection
partial sums are reduced with a chunked ReduceScatter over replica groups
[[0,1,2,3],[4,5,6,7]]; the host concatenates the row shards.

Device-side layout ("transposed attention"):
  xT [D, T] (host pre-transposed, bf16)
  qT/kT [H, T] = proj + rope (rope tables host-precomputed from positions;
                 the H^-0.5 scale is folded into wq on the host)
  logitsT [S-chunk(128), T-blk(512)] = kT-chunk^T @ qT   (PSUM f32)
  expT = exp(logitsT) via ScalarE, bf16; causal masking via 4 static
         diagonal 0/1 tiles; fully-masked chunks are skipped entirely.
  dsum [1, T-blk] += ones^T @ expT (PE), recip via DVE, broadcast across
         partitions with a K=1 ones outer-product matmul, folded into the
         encodedT normalization multiply.
  encT [H, T] += v-chunk^T @ expT
  out [T-tile(128), D-blk(512)] = sum_h encT_h^T @ wo_h  -> bf16 -> RS.

Scheduling notes: engine queues are in-order, so the key loops are
software-pipelined at emission time (logits/exp of chunk j+1 are emitted
before the PV matmuls of chunk j; rope runs one block behind the
projection matmuls). xT is DMA'd in column blocks interleaved with the
weights so the first projections start ~10us in instead of waiting for
all 16 MB of input.
"""

import numpy as np
import ml_dtypes

import concourse.bass as bass
import concourse.bacc as bacc
import concourse.mybir as mybir
import concourse.tile as tile
from concourse import bass_utils

BF = mybir.dt.bfloat16
F32 = mybir.dt.float32

B, T, D, N, KVH, H = 2, 2048, 2048, 8, 1, 256
MAX_WAVELENGTH = 10000
TBLK = 512    # T block (matmul moving free dim / PSUM bank)
SCH = 128     # S chunk (key chunk, PSUM partition dim)
TT = 128      # T tile (out-projection partition dim)
RS_ROWS = 256  # rows per ReduceScatter chunk
GROUPS = [[0, 1, 2, 3], [4, 5, 6, 7]]
N_CORES = 8


def rs_regions(t):
    """Phase-B regions == ReduceScatter chunks: 512-wide blocks, the last
    block split into two 256-wide regions."""
    n_tblk = t // TBLK
    regions = [(m * TBLK, TBLK) for m in range(n_tblk - 1)]
    last0 = (n_tblk - 1) * TBLK
    return regions + [(last0, TBLK // 2), (last0 + TBLK // 2, TBLK // 2)]


def build(causal=True, t=T, d=D):
    """Build the SPMD graph (identical on all 8 cores)."""
    n_tblk = t // TBLK
    n_dch = d // 128
    n_dblk = d // TBLK
    n_sch = t // SCH
    n_tt = TBLK // TT
    n_rs = t // RS_ROWS           # RS chunks
    rs_out = RS_ROWS // 4         # rows per core per RS chunk

    nc = bacc.Bacc("TRN2", target_bir_lowering=False, debug=False,
                   num_devices=N_CORES)

    xT_e = nc.dram_tensor("xT", [d, t], BF, kind="ExternalInput")
    wq_e = nc.dram_tensor("wq", [d, 2 * H], BF, kind="ExternalInput")
    wk_e = nc.dram_tensor("wk", [d, H], BF, kind="ExternalInput")
    wv_e = nc.dram_tensor("wv", [d, H], BF, kind="ExternalInput")
    wo_e = nc.dram_tensor("wo", [2 * H, d], BF, kind="ExternalInput")
    cos_e = nc.dram_tensor("cosT", [H // 2, t], F32, kind="ExternalInput")
    sin_e = nc.dram_tensor("sinT", [H // 2, t], F32, kind="ExternalInput")
    if causal:
        cm_e = nc.dram_tensor("cmask", [SCH, 4 * TBLK], BF, kind="ExternalInput")
    else:
        gm_e = nc.dram_tensor("gmask", [t, t], BF, kind="ExternalInput")
    out_e = nc.dram_tensor("out", [t // 4, d], BF, kind="ExternalOutput")

    with tile.TileContext(nc) as tc:
        poolP = tc.alloc_tile_pool(name="persist", bufs=1)
        poolT = tc.alloc_tile_pool(name="tmps", bufs=4)
        poolPS = tc.alloc_tile_pool(name="ps", bufs=1, space="PSUM")
        poolD = tc.alloc_tile_pool(name="dram", bufs=1, space="DRAM")
        poolW = tc.alloc_tile_pool(name="w", bufs=1)

        # ---- input loads ------------------------------------------------
        # one wide SBUF tile per tensor; batched 3D-AP DMAs (partition p,
        # D-chunk di, column c) keep the Sync trigger count low — per-DMA
        # trigger dispatch (~0.6us each) was the startup bottleneck
        x_sb = poolW.tile([128, n_dch * t], BF, name="x_sb")
        wq_sb = poolW.tile([128, n_dch * 2 * H], BF, name="wq_sb")
        wk_sb = poolW.tile([128, n_dch * H], BF, name="wk_sb")
        wv_sb = poolW.tile([128, n_dch * H], BF, name="wv_sb")
        xts = [x_sb[:, i * t:(i + 1) * t] for i in range(n_dch)]
        wqs = [wq_sb[:, i * 2 * H:(i + 1) * 2 * H] for i in range(n_dch)]
        wks = [wk_sb[:, i * H:(i + 1) * H] for i in range(n_dch)]
        wvs = [wv_sb[:, i * H:(i + 1) * H] for i in range(n_dch)]
        wos = [poolP.tile([128, d], BF, name=f"wot{k}") for k in range(4)]
        cos_sb = poolP.tile([128, t], F32, name="cos_sb")
        sin_sb = poolP.tile([128, t], F32, name="sin_sb")

        xT_r = xT_e.ap().rearrange("(i p) t -> p i t", p=128)
        x_sb3 = x_sb.rearrange("p (i t) -> p i t", i=n_dch)

        def load_x_cols(c0, c1, n_split=2):
            # split the D-chunk axis so consumers unblock progressively
            step = n_dch // n_split
            for s in range(n_split):
                i0, i1 = s * step, (s + 1) * step
                nc.sync.dma_start(x_sb3[:, i0:i1, c0:c1], xT_r[:, i0:i1, c0:c1])

        def load_w(dst, src, cols, n_split=2):
            src_r = src.ap().rearrange("(i p) c -> p i c", p=128)
            dst_r = dst.rearrange("p (i c) -> p i c", i=n_dch)
            step = n_dch // n_split
            for s in range(n_split):
                i0, i1 = s * step, (s + 1) * step
                nc.sync.dma_start(dst_r[:, i0:i1, :], src_r[:, i0:i1, :])

        load_w(wv_sb, wv_e, H)
        load_x_cols(0, TBLK, n_split=4)
        nc.sync.dma_start(cos_sb[:], cos_e.ap()[:, :])
        nc.sync.dma_start(sin_sb[:], sin_e.ap()[:, :])
        load_w(wk_sb, wk_e, H)
        load_w(wq_sb, wq_e, 2 * H)
        if causal:
            cm_sb = poolP.tile([SCH, 4 * TBLK], BF, name="cm_sb")
            nc.sync.dma_start(cm_sb[:], cm_e.ap()[:, :])
        if n_tblk > 1:
            load_x_cols(TBLK, t, n_split=4)
        for k in range(4):
            nc.sync.dma_start(wos[k][:], wo_e.ap()[128 * k:128 * (k + 1), :])

        ones_col = poolP.tile([128, 1], BF, name="ones_col")
        nc.vector.memset(ones_col[:], 1.0)

        # ---- phase A: projections + rope, interleaved per T block so the
        # matmuls chase the xT column-block DMAs and phase B's early blocks
        # unblock as soon as possible
        v_sb = [poolP.tile([128, H], BF, name=f"v{j}") for j in range(n_sch)]
        ktop = poolP.tile([128, t], BF, name="ktop")
        kbot = poolP.tile([128, t], BF, name="kbot")
        qtop = [poolP.tile([128, t], BF, name=f"qtop{h}") for h in range(2)]
        qbot = [poolP.tile([128, t], BF, name=f"qbot{h}") for h in range(2)]

        def emit_proj(w_tiles, col0, m):
            sl = slice(m * TBLK, (m + 1) * TBLK)
            ps_top = poolPS.tile([128, TBLK], F32, name="ps_top", tag="qk", bufs=2)
            ps_bot = poolPS.tile([128, TBLK], F32, name="ps_bot", tag="enc", bufs=2)
            for di in range(n_dch):
                nc.tensor.matmul(ps_top[:], w_tiles[di][:, col0:col0 + 128],
                                 xts[di][:, sl], start=(di == 0),
                                 stop=(di == n_dch - 1))
            for di in range(n_dch):
                nc.tensor.matmul(ps_bot[:], w_tiles[di][:, col0 + 128:col0 + 256],
                                 xts[di][:, sl], start=(di == 0),
                                 stop=(di == n_dch - 1))
            return ps_top, ps_bot

        def emit_rope(job):
            top_dst, bot_dst, m, ps_top, ps_bot = job
            sl = slice(m * TBLK, (m + 1) * TBLK)
            c_sl, s_sl = cos_sb[:, sl], sin_sb[:, sl]
            t1 = poolT.tile([128, TBLK], F32, name="rt1", tag="tmp")
            t2 = poolT.tile([128, TBLK], F32, name="rt2", tag="tmp")
            nc.vector.tensor_mul(t1[:], ps_top[:], c_sl)
            nc.vector.tensor_mul(t2[:], ps_bot[:], s_sl)
            nc.vector.tensor_sub(top_dst[:, sl], t1[:], t2[:])
            t3 = poolT.tile([128, TBLK], F32, name="rt3", tag="tmp")
            t4 = poolT.tile([128, TBLK], F32, name="rt4", tag="tmp")
            nc.vector.tensor_mul(t3[:], ps_bot[:], c_sl)
            nc.vector.tensor_mul(t4[:], ps_top[:], s_sl)
            nc.vector.tensor_add(bot_dst[:, sl], t3[:], t4[:])

        # rope runs one projection block behind so the PE never waits on DVE
        pending = None
        for m in range(n_tblk):
            for j in range(4 * m, 4 * m + 4):
                ps_v = poolPS.tile([128, H], F32, name="ps_v",
                                   tag="wo" if j % 2 == 0 else "aux", bufs=2)
                for di in range(n_dch):
                    nc.tensor.matmul(ps_v[:], xts[di][:, j * SCH:(j + 1) * SCH],
                                     wvs[di][:], start=(di == 0),
                                     stop=(di == n_dch - 1))
                nc.vector.tensor_copy(v_sb[j][:], ps_v[:])
            for (top_dst, bot_dst, w_tiles, col0) in (
                    (ktop, kbot, wks, 0),
                    (qtop[0], qbot[0], wqs, 0),
                    (qtop[1], qbot[1], wqs, H)):
                ps_top, ps_bot = emit_proj(w_tiles, col0, m)
                if pending is not None:
                    emit_rope(pending)
                pending = (top_dst, bot_dst, m, ps_top, ps_bot)
        emit_rope(pending)

        poolW.release()
        poolB = tc.alloc_tile_pool(name="phaseB", bufs=1)
        if not causal:
            poolG = tc.alloc_tile_pool(name="gmask", bufs=4)

        encT = [poolB.tile([128, t], BF, name=f"enc{k}") for k in range(4)]
        in_bounce = poolD.tile([t, d], BF, name="in_bounce")
        out_bounces = {}

        # ---- phase B: attention + out-projection + chunked RS -----------
        # a region is (t0, tw) — tw columns starting at t0. Normal blocks are
        # 512 wide; the final block is split into two 256-wide regions so the
        # last ReduceScatter chunk is small and lands early.
        def emit_attention(t0, tw, h):
            t_sl = slice(t0, t0 + tw)
            n_chunks = (t0 + tw) // SCH if causal else n_sch
            ps_e0 = poolPS.tile([128, tw], F32, name="ps_e0", tag="enc", bufs=2)
            ps_e1 = poolPS.tile([128, tw], F32, name="ps_e1", tag="enc", bufs=2)
            ps_ds = poolPS.tile([1, tw], F32, name="ps_ds", tag="aux", bufs=2)

            def emit_logits_exp(j):
                s_sl = slice(j * SCH, (j + 1) * SCH)
                ps_l = poolPS.tile([128, tw], F32, name="ps_l", tag="qk", bufs=2)
                nc.tensor.matmul(ps_l[:], ktop[:, s_sl], qtop[h][:, t_sl],
                                 start=True, stop=False)
                nc.tensor.matmul(ps_l[:], kbot[:, s_sl], qbot[h][:, t_sl],
                                 start=False, stop=True)
                ex = poolB.tile([128, TBLK], BF, name="ex", tag="ex", bufs=8)
                nc.scalar.activation(ex[:, :tw], ps_l[:],
                                     mybir.ActivationFunctionType.Exp)
                if causal:
                    if j >= t0 // SCH:
                        i = j - t0 // SCH
                        nc.vector.tensor_mul(
                            ex[:, :tw], ex[:, :tw],
                            cm_sb[:, i * TBLK:i * TBLK + tw])
                else:
                    gm = poolG.tile([128, TBLK], BF, name="gm", tag="gm")
                    nc.sync.dma_start(gm[:, :tw], gm_e.ap()[s_sl, t_sl])
                    nc.vector.tensor_mul(ex[:, :tw], ex[:, :tw], gm[:, :tw])
                return ex

            # software pipeline, 2 deep: logits/exp of j+2 issue before PV of
            # j, so the ScalarE exp latency is fully hidden (qk bufs=2 holds
            # exactly ps_l(j+1) and ps_l(j+2) once exp(j) frees its bank)
            ex_q = [emit_logits_exp(jj) for jj in range(min(2, n_chunks))]
            for j in range(n_chunks):
                ex = ex_q.pop(0)
                if j + 2 < n_chunks:
                    ex_q.append(emit_logits_exp(j + 2))
                last = j == n_chunks - 1
                nc.tensor.matmul(ps_e0[:], v_sb[j][:, 0:128], ex[:, :tw],
                                 start=(j == 0), stop=last)
                nc.tensor.matmul(ps_e1[:], v_sb[j][:, 128:256], ex[:, :tw],
                                 start=(j == 0), stop=last)
                nc.tensor.matmul(ps_ds[:], ones_col[:], ex[:, :tw],
                                 start=(j == 0), stop=last)

            # free the enc PSUM banks immediately; reciprocal + partition
            # broadcast run off the PE queue. The encT normalize multiplies
            # are deferred to the consuming wo so they never head-of-line
            # block the DVE queue in front of the next head's mask muls.
            ef0 = poolB.tile([128, TBLK], F32, name="ef0", tag="ef", bufs=8)
            ef1 = poolB.tile([128, TBLK], F32, name="ef1", tag="ef", bufs=8)
            nc.vector.tensor_copy(ef0[:, :tw], ps_e0[:])
            nc.vector.tensor_copy(ef1[:, :tw], ps_e1[:])
            rrow = poolB.tile([1, TBLK], F32, name="rrow", tag="rrow", bufs=4)
            nc.vector.reciprocal_approx_fast(rrow[:, :tw], ps_ds[:])
            rbc = poolB.tile([128, TBLK], F32, name="rbc", tag="rbc", bufs=4)
            nc.gpsimd.partition_broadcast(rbc[:, :tw], rrow[:, :tw])
            return (ef0, ef1, rbc, t_sl, tw, h)

        def emit_norm(job):
            ef0, ef1, rbc, t_sl, tw, h = job
            nc.vector.tensor_mul(encT[2 * h][:, t_sl], ef0[:, :tw], rbc[:, :tw])
            nc.vector.tensor_mul(encT[2 * h + 1][:, t_sl], ef1[:, :tw], rbc[:, :tw])

        def emit_wo_rs(t0, tw):
            # out-projection for region; RS every RS_ROWS rows
            for tt in range(tw // TT):
                r_sl = slice(t0 + tt * TT, t0 + (tt + 1) * TT)
                for k_db in range(n_dblk):
                    d_sl = slice(k_db * TBLK, (k_db + 1) * TBLK)
                    ps_o = poolPS.tile([128, TBLK], F32, name="ps_o", tag="wo", bufs=2)
                    for k in range(4):
                        nc.tensor.matmul(ps_o[:], encT[k][:, r_sl],
                                         wos[k][:, d_sl], start=(k == 0),
                                         stop=(k == 3))
                    ostg = poolB.tile([128, TBLK], BF, name="ostg", tag="ostg", bufs=6)
                    nc.vector.tensor_copy(ostg[:], ps_o[:])
                    last_dma[0] = nc.sync.dma_start(in_bounce[r_sl, d_sl], ostg[:])
            # one RS per region: big chunks early (amortizes the per-RS
            # floor), the small last-block regions keep the tail short
            ob = poolD.tile([tw // 4, d], BF, name=f"out_b{t0}")
            out_bounces[t0] = ob
            nc.gpsimd.collective_compute(
                "ReduceScatter", mybir.AluOpType.add,
                replica_groups=GROUPS,
                ins=[in_bounce[t0:t0 + tw, :].opt()],
                outs=[ob.opt()])
            rs_done.append((t0, tw))

        regions = rs_regions(t)
        rs_done = []
        last_dma = [None]
        wo_pending = None
        for (t0, tw) in regions:
            j0 = emit_attention(t0, tw, 0)
            if wo_pending is not None:
                pt0, ptw, jobs = wo_pending
                emit_norm(jobs[0])
                emit_norm(jobs[1])
                emit_wo_rs(pt0, ptw)
            j1 = emit_attention(t0, tw, 1)
            wo_pending = (t0, tw, [j0, j1])
        pt0, ptw, jobs = wo_pending
        emit_norm(jobs[0])
        emit_norm(jobs[1])
        emit_wo_rs(pt0, ptw)
        # output drains last, on the gpsimd queue: an out_e DMA stalls on its
        # RS, and anywhere earlier that wait would head-of-line block the
        # queue. The scheduler hoists by modeled readiness, so fence first.
        tc.no_sync_barrier()
        for (t0, tw) in rs_done:
            nc.gpsimd.dma_start(
                out_e.ap()[t0 // 4:(t0 + tw) // 4, :], out_bounces[t0][:])

        if not causal:
            poolG.release()
        poolB.release()
        poolD.release()
        poolPS.release()
        poolT.release()
        poolP.release()

    nc.compile()
    return nc


_NC_CACHE = {}


def _get_nc(causal, t=T, d=D):
    key = (causal, t, d)
    if key not in _NC_CACHE:
        _NC_CACHE[key] = build(causal, t, d)
    return _NC_CACHE[key]


def _rope_tables(pos):
    """pos [T] f32 -> cosT, sinT [H/2, T] f32."""
    half = H // 2
    freq_exp = (2.0 / H) * np.arange(half, dtype=np.float32)
    timescale = (MAX_WAVELENGTH ** freq_exp).astype(np.float32)
    radians = pos[None, :].astype(np.float32) / timescale[:, None]
    return np.cos(radians).astype(np.float32), np.sin(radians).astype(np.float32)


def _causal_tiles():
    """4 diagonal 0/1 tiles [SCH, TBLK]: tile i -> 1{ds + 128*i <= dt}."""
    ds = np.arange(SCH)[:, None]
    dt = np.arange(TBLK)[None, :]
    tiles = [(dt >= ds + SCH * i).astype(np.float32) for i in range(4)]
    return np.concatenate(tiles, axis=1).astype(ml_dtypes.bfloat16)


def _prep_in_maps(x, positions, attn_mask, wq, wkv, wo, causal):
    bf = ml_dtypes.bfloat16
    scale = np.float32(H) ** np.float32(-0.5)
    wq_s = (np.asarray(wq, np.float32) * scale)
    wk = np.asarray(wkv[0, 0], np.float32).astype(bf)
    wv = np.asarray(wkv[1, 0], np.float32).astype(bf)
    cm = _causal_tiles() if causal else None

    in_maps = []
    for c in range(N_CORES):
        b, r = divmod(c, 4)
        h0, h1 = 2 * r, 2 * r + 1
        xT = np.ascontiguousarray(np.asarray(x[b], np.float32).T).astype(bf)
        wq_c = np.ascontiguousarray(
            np.concatenate([wq_s[h0], wq_s[h1]], axis=1)).astype(bf)
        wo_c = np.ascontiguousarray(
            np.concatenate([np.asarray(wo[h0], np.float32),
                            np.asarray(wo[h1], np.float32)], axis=0)).astype(bf)
        cosT, sinT = _rope_tables(np.asarray(positions[b], np.float32))
        m = {"xT": xT, "wq": wq_c, "wk": wk, "wv": wv, "wo": wo_c,
             "cosT": cosT, "sinT": sinT}
        if causal:
            m["cmask"] = cm
        else:
            m["gmask"] = np.ascontiguousarray(
                np.asarray(attn_mask[b, 0], np.float32).T).astype(bf)
        in_maps.append(m)
    return in_maps


def kernel(x, positions, attn_mask, wq, wkv, wo):
    x = np.asarray(x)
    positions = np.asarray(positions)
    attn_mask = np.asarray(attn_mask)
    wq, wkv, wo = np.asarray(wq), np.asarray(wkv), np.asarray(wo)

    tril = np.tril(np.ones((T, T), bool))
    causal = all(np.array_equal(attn_mask[b, 0], tril) for b in range(B))

    nc = _get_nc(causal)
    in_maps = _prep_in_maps(x, positions, attn_mask, wq, wkv, wo, causal)
    res = bass_utils.run_bass_kernel_spmd(nc, in_maps,
                                          core_ids=list(range(N_CORES)))

    out = np.empty((B, T, D), np.float32)
    for c in range(N_CORES):
        b, r = divmod(c, 4)
        shard = np.asarray(res.results[c]["out"], dtype=np.float32)
        for (t0, tw) in rs_regions(T):
            rows = tw // 4
            out[b, t0 + r * rows:t0 + (r + 1) * rows, :] = \
                shard[t0 // 4:t0 // 4 + rows, :]
    return out


# revision 32
# speedup vs baseline: 1.0347x; 1.0347x over previous
"""Distributed Trainium2 kernel for GQA attention (B=2, T=2048, D=2048, N=8
query heads, K=1 KV head, H=256) on 8 NeuronCores.

Sharding: 2 (batch) x 4 (head-pair) mesh. Core c handles batch c//4 and query
heads {2*(c%4), 2*(c%4)+1}. K/V projections are computed per batch group
(replicated across the 4 cores of a group). The per-head out-projection
partial sums are reduced with a chunked ReduceScatter over replica groups
[[0,1,2,3],[4,5,6,7]]; the host concatenates the row shards.

Device-side layout ("transposed attention"):
  xT [D, T] (host pre-transposed, bf16)
  qT/kT [H, T] = proj + rope (rope tables host-precomputed from positions;
                 the H^-0.5 scale is folded into wq on the host)
  logitsT [S-chunk(128), T-blk(512)] = kT-chunk^T @ qT   (PSUM f32)
  expT = exp(logitsT) via ScalarE, bf16; causal masking via 4 static
         diagonal 0/1 tiles; fully-masked chunks are skipped entirely.
  dsum [1, T-blk] += ones^T @ expT (PE), recip via DVE, broadcast across
         partitions with a K=1 ones outer-product matmul, folded into the
         encodedT normalization multiply.
  encT [H, T] += v-chunk^T @ expT
  out [T-tile(128), D-blk(512)] = sum_h encT_h^T @ wo_h  -> bf16 -> RS.

Scheduling notes: engine queues are in-order, so the key loops are
software-pipelined at emission time (logits/exp of chunk j+1 are emitted
before the PV matmuls of chunk j; rope runs one block behind the
projection matmuls). xT is DMA'd in column blocks interleaved with the
weights so the first projections start ~10us in instead of waiting for
all 16 MB of input.
"""

import numpy as np
import ml_dtypes

import concourse.bass as bass
import concourse.bacc as bacc
import concourse.mybir as mybir
import concourse.tile as tile
from concourse import bass_utils

BF = mybir.dt.bfloat16
F32 = mybir.dt.float32

B, T, D, N, KVH, H = 2, 2048, 2048, 8, 1, 256
MAX_WAVELENGTH = 10000
TBLK = 512    # T block (matmul moving free dim / PSUM bank)
SCH = 128     # S chunk (key chunk, PSUM partition dim)
TT = 128      # T tile (out-projection partition dim)
RS_ROWS = 256  # rows per ReduceScatter chunk
GROUPS = [[0, 1, 2, 3], [4, 5, 6, 7]]
N_CORES = 8


def rs_regions(t):
    """Phase-B regions == ReduceScatter chunks: 512-wide blocks, the last
    block split into two 256-wide regions."""
    n_tblk = t // TBLK
    regions = [(m * TBLK, TBLK) for m in range(n_tblk - 1)]
    last0 = (n_tblk - 1) * TBLK
    return regions + [(last0, TBLK // 2), (last0 + TBLK // 2, TBLK // 2)]


def build(causal=True, t=T, d=D):
    """Build the SPMD graph (identical on all 8 cores)."""
    n_tblk = t // TBLK
    n_dch = d // 128
    n_dblk = d // TBLK
    n_sch = t // SCH
    n_tt = TBLK // TT
    n_rs = t // RS_ROWS           # RS chunks
    rs_out = RS_ROWS // 4         # rows per core per RS chunk

    nc = bacc.Bacc("TRN2", target_bir_lowering=False, debug=False,
                   num_devices=N_CORES)

    xT_e = nc.dram_tensor("xT", [d, t], BF, kind="ExternalInput")
    wq_e = nc.dram_tensor("wq", [d, 2 * H], BF, kind="ExternalInput")
    wk_e = nc.dram_tensor("wk", [d, H], BF, kind="ExternalInput")
    wv_e = nc.dram_tensor("wv", [d, H], BF, kind="ExternalInput")
    wo_e = nc.dram_tensor("wo", [2 * H, d], BF, kind="ExternalInput")
    cos_e = nc.dram_tensor("cosT", [H // 2, t], F32, kind="ExternalInput")
    sin_e = nc.dram_tensor("sinT", [H // 2, t], F32, kind="ExternalInput")
    if causal:
        cm_e = nc.dram_tensor("cmask", [SCH, 4 * TBLK], BF, kind="ExternalInput")
    else:
        gm_e = nc.dram_tensor("gmask", [t, t], BF, kind="ExternalInput")
    out_e = nc.dram_tensor("out", [t // 4, d], BF, kind="ExternalOutput")

    with tile.TileContext(nc) as tc:
        poolP = tc.alloc_tile_pool(name="persist", bufs=1)
        poolT = tc.alloc_tile_pool(name="tmps", bufs=4)
        poolPS = tc.alloc_tile_pool(name="ps", bufs=1, space="PSUM")
        poolD = tc.alloc_tile_pool(name="dram", bufs=1, space="DRAM")
        poolW = tc.alloc_tile_pool(name="w", bufs=1)

        # ---- input loads ------------------------------------------------
        # one wide SBUF tile per tensor; batched 3D-AP DMAs (partition p,
        # D-chunk di, column c) keep the Sync trigger count low — per-DMA
        # trigger dispatch (~0.6us each) was the startup bottleneck
        x_sb = poolW.tile([128, n_dch * t], BF, name="x_sb")
        wq_sb = poolW.tile([128, n_dch * 2 * H], BF, name="wq_sb")
        wk_sb = poolW.tile([128, n_dch * H], BF, name="wk_sb")
        wv_sb = poolW.tile([128, n_dch * H], BF, name="wv_sb")
        xts = [x_sb[:, i * t:(i + 1) * t] for i in range(n_dch)]
        wqs = [wq_sb[:, i * 2 * H:(i + 1) * 2 * H] for i in range(n_dch)]
        wks = [wk_sb[:, i * H:(i + 1) * H] for i in range(n_dch)]
        wvs = [wv_sb[:, i * H:(i + 1) * H] for i in range(n_dch)]
        wos = [poolP.tile([128, d], BF, name=f"wot{k}") for k in range(4)]
        cos_sb = poolP.tile([128, t], F32, name="cos_sb")
        sin_sb = poolP.tile([128, t], F32, name="sin_sb")

        xT_r = xT_e.ap().rearrange("(i p) t -> p i t", p=128)
        x_sb3 = x_sb.rearrange("p (i t) -> p i t", i=n_dch)

        def load_x_cols(c0, c1, n_split=2):
            # split the D-chunk axis so consumers unblock progressively
            step = n_dch // n_split
            for s in range(n_split):
                i0, i1 = s * step, (s + 1) * step
                nc.sync.dma_start(x_sb3[:, i0:i1, c0:c1], xT_r[:, i0:i1, c0:c1])

        def load_w(dst, src, cols, n_split=2):
            src_r = src.ap().rearrange("(i p) c -> p i c", p=128)
            dst_r = dst.rearrange("p (i c) -> p i c", i=n_dch)
            step = n_dch // n_split
            for s in range(n_split):
                i0, i1 = s * step, (s + 1) * step
                nc.sync.dma_start(dst_r[:, i0:i1, :], src_r[:, i0:i1, :])

        load_w(wv_sb, wv_e, H)
        load_x_cols(0, TBLK, n_split=4)
        nc.sync.dma_start(cos_sb[:], cos_e.ap()[:, :])
        nc.sync.dma_start(sin_sb[:], sin_e.ap()[:, :])
        load_w(wk_sb, wk_e, H)
        load_w(wq_sb, wq_e, 2 * H)
        if causal:
            cm_sb = poolP.tile([SCH, 4 * TBLK], BF, name="cm_sb")
            nc.sync.dma_start(cm_sb[:], cm_e.ap()[:, :])
        if n_tblk > 1:
            load_x_cols(TBLK, t, n_split=4)
        for k in range(4):
            nc.sync.dma_start(wos[k][:], wo_e.ap()[128 * k:128 * (k + 1), :])

        ones_col = poolP.tile([128, 1], BF, name="ones_col")
        nc.vector.memset(ones_col[:], 1.0)

        # ---- phase A: projections + rope, interleaved per T block so the
        # matmuls chase the xT column-block DMAs and phase B's early blocks
        # unblock as soon as possible
        v_sb = [poolP.tile([128, H], BF, name=f"v{j}") for j in range(n_sch)]
        ktop = poolP.tile([128, t], BF, name="ktop")
        kbot = poolP.tile([128, t], BF, name="kbot")
        qtop = [poolP.tile([128, t], BF, name=f"qtop{h}") for h in range(2)]
        qbot = [poolP.tile([128, t], BF, name=f"qbot{h}") for h in range(2)]

        def emit_proj(w_tiles, col0, m):
            sl = slice(m * TBLK, (m + 1) * TBLK)
            ps_top = poolPS.tile([128, TBLK], F32, name="ps_top", tag="qk", bufs=2)
            ps_bot = poolPS.tile([128, TBLK], F32, name="ps_bot", tag="enc", bufs=2)
            for di in range(n_dch):
                nc.tensor.matmul(ps_top[:], w_tiles[di][:, col0:col0 + 128],
                                 xts[di][:, sl], start=(di == 0),
                                 stop=(di == n_dch - 1))
            for di in range(n_dch):
                nc.tensor.matmul(ps_bot[:], w_tiles[di][:, col0 + 128:col0 + 256],
                                 xts[di][:, sl], start=(di == 0),
                                 stop=(di == n_dch - 1))
            return ps_top, ps_bot

        def emit_rope(job):
            top_dst, bot_dst, m, ps_top, ps_bot = job
            sl = slice(m * TBLK, (m + 1) * TBLK)
            c_sl, s_sl = cos_sb[:, sl], sin_sb[:, sl]
            t1 = poolT.tile([128, TBLK], F32, name="rt1", tag="tmp")
            t2 = poolT.tile([128, TBLK], F32, name="rt2", tag="tmp")
            nc.vector.tensor_mul(t1[:], ps_top[:], c_sl)
            nc.vector.tensor_mul(t2[:], ps_bot[:], s_sl)
            nc.vector.tensor_sub(top_dst[:, sl], t1[:], t2[:])
            t3 = poolT.tile([128, TBLK], F32, name="rt3", tag="tmp")
            t4 = poolT.tile([128, TBLK], F32, name="rt4", tag="tmp")
            nc.vector.tensor_mul(t3[:], ps_bot[:], c_sl)
            nc.vector.tensor_mul(t4[:], ps_top[:], s_sl)
            nc.vector.tensor_add(bot_dst[:, sl], t3[:], t4[:])

        # rope runs one projection block behind so the PE never waits on DVE
        pending = None
        for m in range(n_tblk):
            for j in range(4 * m, 4 * m + 4):
                ps_v = poolPS.tile([128, H], F32, name="ps_v",
                                   tag="wo" if j % 2 == 0 else "aux", bufs=2)
                for di in range(n_dch):
                    nc.tensor.matmul(ps_v[:], xts[di][:, j * SCH:(j + 1) * SCH],
                                     wvs[di][:], start=(di == 0),
                                     stop=(di == n_dch - 1))
                nc.vector.tensor_copy(v_sb[j][:], ps_v[:])
            for (top_dst, bot_dst, w_tiles, col0) in (
                    (ktop, kbot, wks, 0),
                    (qtop[0], qbot[0], wqs, 0),
                    (qtop[1], qbot[1], wqs, H)):
                ps_top, ps_bot = emit_proj(w_tiles, col0, m)
                if pending is not None:
                    emit_rope(pending)
                pending = (top_dst, bot_dst, m, ps_top, ps_bot)
        emit_rope(pending)

        poolW.release()
        poolB = tc.alloc_tile_pool(name="phaseB", bufs=1)
        if not causal:
            poolG = tc.alloc_tile_pool(name="gmask", bufs=4)

        encT = [poolB.tile([128, t], BF, name=f"enc{k}") for k in range(4)]
        in_bounce = poolD.tile([t, d], BF, name="in_bounce")
        out_bounces = {}

        # ---- phase B: attention + out-projection + chunked RS -----------
        # a region is (t0, tw) — tw columns starting at t0. Normal blocks are
        # 512 wide; the final block is split into two 256-wide regions so the
        # last ReduceScatter chunk is small and lands early.
        def emit_attention(t0, tw, h, mid_hook=None):
            t_sl = slice(t0, t0 + tw)
            n_chunks = (t0 + tw) // SCH if causal else n_sch
            ps_e0 = poolPS.tile([128, tw], F32, name="ps_e0", tag="enc", bufs=2)
            ps_e1 = poolPS.tile([128, tw], F32, name="ps_e1", tag="enc", bufs=2)
            ps_ds = poolPS.tile([1, tw], F32, name="ps_ds", tag="aux", bufs=2)

            def emit_logits_exp(j):
                s_sl = slice(j * SCH, (j + 1) * SCH)
                ps_l = poolPS.tile([128, tw], F32, name="ps_l", tag="qk", bufs=2)
                nc.tensor.matmul(ps_l[:], ktop[:, s_sl], qtop[h][:, t_sl],
                                 start=True, stop=False)
                nc.tensor.matmul(ps_l[:], kbot[:, s_sl], qbot[h][:, t_sl],
                                 start=False, stop=True)
                ex = poolB.tile([128, TBLK], BF, name="ex", tag="ex", bufs=8)
                nc.scalar.activation(ex[:, :tw], ps_l[:],
                                     mybir.ActivationFunctionType.Exp)
                if causal:
                    if j >= t0 // SCH:
                        i = j - t0 // SCH
                        nc.vector.tensor_mul(
                            ex[:, :tw], ex[:, :tw],
                            cm_sb[:, i * TBLK:i * TBLK + tw])
                else:
                    gm = poolG.tile([128, TBLK], BF, name="gm", tag="gm")
                    nc.sync.dma_start(gm[:, :tw], gm_e.ap()[s_sl, t_sl])
                    nc.vector.tensor_mul(ex[:, :tw], ex[:, :tw], gm[:, :tw])
                return ex

            # software pipeline, 2 deep: logits/exp of j+2 issue before PV of
            # j, so the ScalarE exp latency is fully hidden (qk bufs=2 holds
            # exactly ps_l(j+1) and ps_l(j+2) once exp(j) frees its bank)
            ex_q = [emit_logits_exp(jj) for jj in range(min(2, n_chunks))]
            for j in range(n_chunks):
                if j == min(3, n_chunks - 1) and mid_hook is not None:
                    mid_hook()  # pending wo/RS lands here: a few chunks of
                    mid_hook = None  # PE runway cover the norm-chain latency
                ex = ex_q.pop(0)
                if j + 2 < n_chunks:
                    ex_q.append(emit_logits_exp(j + 2))
                last = j == n_chunks - 1
                nc.tensor.matmul(ps_e0[:], v_sb[j][:, 0:128], ex[:, :tw],
                                 start=(j == 0), stop=last)
                nc.tensor.matmul(ps_e1[:], v_sb[j][:, 128:256], ex[:, :tw],
                                 start=(j == 0), stop=last)
                nc.tensor.matmul(ps_ds[:], ones_col[:], ex[:, :tw],
                                 start=(j == 0), stop=last)

            # free the enc PSUM banks immediately; reciprocal + partition
            # broadcast run off the PE queue. The encT normalize multiplies
            # are deferred to the consuming wo so they never head-of-line
            # block the DVE queue in front of the next head's mask muls.
            ef0 = poolB.tile([128, TBLK], F32, name="ef0", tag="ef", bufs=8)
            ef1 = poolB.tile([128, TBLK], F32, name="ef1", tag="ef", bufs=8)
            nc.vector.tensor_copy(ef0[:, :tw], ps_e0[:])
            nc.vector.tensor_copy(ef1[:, :tw], ps_e1[:])
            rrow = poolB.tile([1, TBLK], F32, name="rrow", tag="rrow", bufs=4)
            nc.vector.reciprocal_approx_fast(rrow[:, :tw], ps_ds[:])
            rbc = poolB.tile([128, TBLK], F32, name="rbc", tag="rbc", bufs=4)
            nc.gpsimd.partition_broadcast(rbc[:, :tw], rrow[:, :tw])
            return (ef0, ef1, rbc, t_sl, tw, h)

        def emit_norm(job):
            ef0, ef1, rbc, t_sl, tw, h = job
            nc.vector.tensor_mul(encT[2 * h][:, t_sl], ef0[:, :tw], rbc[:, :tw])
            nc.vector.tensor_mul(encT[2 * h + 1][:, t_sl], ef1[:, :tw], rbc[:, :tw])

        def emit_wo_rs(t0, tw):
            # out-projection for region; RS every RS_ROWS rows
            for tt in range(tw // TT):
                r_sl = slice(t0 + tt * TT, t0 + (tt + 1) * TT)
                for k_db in range(n_dblk):
                    d_sl = slice(k_db * TBLK, (k_db + 1) * TBLK)
                    ps_o = poolPS.tile([128, TBLK], F32, name="ps_o", tag="wo", bufs=2)
                    for k in range(4):
                        nc.tensor.matmul(ps_o[:], encT[k][:, r_sl],
                                         wos[k][:, d_sl], start=(k == 0),
                                         stop=(k == 3))
                    ostg = poolB.tile([128, TBLK], BF, name="ostg", tag="ostg", bufs=6)
                    nc.vector.tensor_copy(ostg[:], ps_o[:])
                    last_dma[0] = nc.sync.dma_start(in_bounce[r_sl, d_sl], ostg[:])
            # one RS per region: big chunks early (amortizes the per-RS
            # floor), the small last-block regions keep the tail short
            ob = poolD.tile([tw // 4, d], BF, name=f"out_b{t0}")
            out_bounces[t0] = ob
            nc.gpsimd.collective_compute(
                "ReduceScatter", mybir.AluOpType.add,
                replica_groups=GROUPS,
                ins=[in_bounce[t0:t0 + tw, :].opt()],
                outs=[ob.opt()])
            rs_done.append((t0, tw))

        regions = rs_regions(t)
        rs_done = []
        last_dma = [None]
        wo_pending = None

        def flush_pending():
            pt0, ptw, jobs = wo_pending
            emit_norm(jobs[0])
            emit_norm(jobs[1])
            emit_wo_rs(pt0, ptw)

        for (t0, tw) in regions:
            hook = flush_pending if wo_pending is not None else None
            j0 = emit_attention(t0, tw, 0, mid_hook=hook)
            j1 = emit_attention(t0, tw, 1)
            wo_pending = (t0, tw, [j0, j1])
        flush_pending()
        # output drains last, on the gpsimd queue: an out_e DMA stalls on its
        # RS, and anywhere earlier that wait would head-of-line block the
        # queue. The scheduler hoists by modeled readiness, so fence first.
        tc.no_sync_barrier()
        for (t0, tw) in rs_done:
            nc.gpsimd.dma_start(
                out_e.ap()[t0 // 4:(t0 + tw) // 4, :], out_bounces[t0][:])

        if not causal:
            poolG.release()
        poolB.release()
        poolD.release()
        poolPS.release()
        poolT.release()
        poolP.release()

    nc.compile()
    return nc


_NC_CACHE = {}


def _get_nc(causal, t=T, d=D):
    key = (causal, t, d)
    if key not in _NC_CACHE:
        _NC_CACHE[key] = build(causal, t, d)
    return _NC_CACHE[key]


def _rope_tables(pos):
    """pos [T] f32 -> cosT, sinT [H/2, T] f32."""
    half = H // 2
    freq_exp = (2.0 / H) * np.arange(half, dtype=np.float32)
    timescale = (MAX_WAVELENGTH ** freq_exp).astype(np.float32)
    radians = pos[None, :].astype(np.float32) / timescale[:, None]
    return np.cos(radians).astype(np.float32), np.sin(radians).astype(np.float32)


def _causal_tiles():
    """4 diagonal 0/1 tiles [SCH, TBLK]: tile i -> 1{ds + 128*i <= dt}."""
    ds = np.arange(SCH)[:, None]
    dt = np.arange(TBLK)[None, :]
    tiles = [(dt >= ds + SCH * i).astype(np.float32) for i in range(4)]
    return np.concatenate(tiles, axis=1).astype(ml_dtypes.bfloat16)


def _prep_in_maps(x, positions, attn_mask, wq, wkv, wo, causal):
    bf = ml_dtypes.bfloat16
    scale = np.float32(H) ** np.float32(-0.5)
    wq_s = (np.asarray(wq, np.float32) * scale)
    wk = np.asarray(wkv[0, 0], np.float32).astype(bf)
    wv = np.asarray(wkv[1, 0], np.float32).astype(bf)
    cm = _causal_tiles() if causal else None

    in_maps = []
    for c in range(N_CORES):
        b, r = divmod(c, 4)
        h0, h1 = 2 * r, 2 * r + 1
        xT = np.ascontiguousarray(np.asarray(x[b], np.float32).T).astype(bf)
        wq_c = np.ascontiguousarray(
            np.concatenate([wq_s[h0], wq_s[h1]], axis=1)).astype(bf)
        wo_c = np.ascontiguousarray(
            np.concatenate([np.asarray(wo[h0], np.float32),
                            np.asarray(wo[h1], np.float32)], axis=0)).astype(bf)
        cosT, sinT = _rope_tables(np.asarray(positions[b], np.float32))
        m = {"xT": xT, "wq": wq_c, "wk": wk, "wv": wv, "wo": wo_c,
             "cosT": cosT, "sinT": sinT}
        if causal:
            m["cmask"] = cm
        else:
            m["gmask"] = np.ascontiguousarray(
                np.asarray(attn_mask[b, 0], np.float32).T).astype(bf)
        in_maps.append(m)
    return in_maps


def kernel(x, positions, attn_mask, wq, wkv, wo):
    x = np.asarray(x)
    positions = np.asarray(positions)
    attn_mask = np.asarray(attn_mask)
    wq, wkv, wo = np.asarray(wq), np.asarray(wkv), np.asarray(wo)

    tril = np.tril(np.ones((T, T), bool))
    causal = all(np.array_equal(attn_mask[b, 0], tril) for b in range(B))

    nc = _get_nc(causal)
    in_maps = _prep_in_maps(x, positions, attn_mask, wq, wkv, wo, causal)
    res = bass_utils.run_bass_kernel_spmd(nc, in_maps,
                                          core_ids=list(range(N_CORES)))

    out = np.empty((B, T, D), np.float32)
    for c in range(N_CORES):
        b, r = divmod(c, 4)
        shard = np.asarray(res.results[c]["out"], dtype=np.float32)
        for (t0, tw) in rs_regions(T):
            rows = tw // 4
            out[b, t0 + r * rows:t0 + (r + 1) * rows, :] = \
                shard[t0 // 4:t0 // 4 + rows, :]
    return out
